# Initial kernel scaffold
#
"""Trainium2 Bass kernel for nn_Block_23338852286694 (dense transformer block).

Sharding: 8 cores = 4 batches x 2 query-halves. Each core computes the full
block for its 512 query tokens (K/V work over the full 1024-token sequence is
duplicated across the pair of cores sharing a batch; no collectives).

On-chip dataflow is feature-major (activations stored transposed, [E, T]):
every linear is out[f_tile, tok] = W[k,f_tile].T @ xT[k, tok] with the natural
weight layout as the stationary operand, so no on-device transposes exist
anywhere. Attention probabilities come out key-major, which feeds the
AV matmul directly with token-major V as stationary; a ones-column appended to
V yields the softmax denominators for free. LayerNorm gains/biases are folded
into the consumer weights on the host; the encoder mask is folded into V by
zeroing masked key rows. All matmuls run in float32r (~11-bit mantissa,
full PE rate).
"""
import numpy as np

import concourse.bass as bass
import concourse.bacc as bacc
import concourse.mybir as mybir
import concourse.tile as tile
from concourse.bass_utils import run_bass_kernel_spmd

F32 = mybir.dt.float32
F32R = mybir.dt.float32r
AF = mybir.ActivationFunctionType
ALU = mybir.AluOpType

B, S, SE = 4, 1024, 1024
E, H, M, D = 768, 12, 100, 64
KC = E // 128            # 6 feature chunks
Q = S // 2               # 512 query tokens per core
EPS = 1e-5
NKT = S // 128           # 8 key tiles

# packed per-partition bias column offsets (see _pack_bias_cols)
BC_Q, BC_K, BC_PROJ, BC_MA, BC_FCQ, BC_FCK, BC_EP, BC_A1, BC_A2, BC_FC, BC_PJ = (
    0, 6, 12, 18, 24, 30, 36, 42, 48, 54, 78)
NB = 84


def _row_bcast_dram(ap, parts):
    """DRAM row [N] -> AP readable as [parts, N] (partition-broadcast)."""
    return bass.AP(tensor=ap.tensor, offset=ap.offset,
                   ap=[[0, parts], list(ap.ap[-1])])


def _row_bcast_sbuf(ap, parts):
    """SBUF [1, N] -> AP [1, parts, N] via step-0 free dim (for DMA)."""
    return bass.AP(tensor=ap.tensor, offset=ap.offset,
                   ap=[[ap.ap[0][0], 1], [0, parts], list(ap.ap[-1])])


def build_program():
    nc = bacc.Bacc(trn_type="TRN2")

    # ---- inputs (per-core; host pre-transposes / pre-folds) ----
    xT = nc.dram_tensor("xT", [E, S], F32R, kind="ExternalInput")
    xqT = nc.dram_tensor("xqT", [E, Q], F32R, kind="ExternalInput")
    encT = nc.dram_tensor("encT", [2, E, SE], F32R, kind="ExternalInput")
    maskmul = nc.dram_tensor("maskmul", [128, NKT], F32, kind="ExternalInput")
    mkT = nc.dram_tensor("mkT", [128, KC, M], F32R, kind="ExternalInput")
    mvA = nc.dram_tensor("mvA", [M, H * 65], F32R, kind="ExternalInput")
    w_qk = nc.dram_tensor("w_qk", [E, 2 * E], F32R, kind="ExternalInput")
    w_vs = nc.dram_tensor("w_vs", [E, E], F32R, kind="ExternalInput")
    w_proj = nc.dram_tensor("w_proj", [E, E], F32R, kind="ExternalInput")
    w_ma = nc.dram_tensor("w_ma", [2 * E, E], F32R, kind="ExternalInput")
    w_q = nc.dram_tensor("w_q", [E, E], F32R, kind="ExternalInput")
    w_k = nc.dram_tensor("w_k", [E, E], F32R, kind="ExternalInput")
    w_v = nc.dram_tensor("w_v", [E, E], F32R, kind="ExternalInput")
    w_ep = nc.dram_tensor("w_ep", [E, E], F32R, kind="ExternalInput")
    w_a1 = nc.dram_tensor("w_a1", [2 * E, E], F32R, kind="ExternalInput")
    w_a2 = nc.dram_tensor("w_a2", [2 * E, E], F32R, kind="ExternalInput")
    w_fc = nc.dram_tensor("w_fc", [E, 4 * E], F32R, kind="ExternalInput")
    w_pj = nc.dram_tensor("w_pj", [4 * E, E], F32R, kind="ExternalInput")
    bcols = nc.dram_tensor("bcols", [128, NB], F32, kind="ExternalInput")
    bv_self = nc.dram_tensor("bv_self", [E], F32, kind="ExternalInput")
    bv_enc = nc.dram_tensor("bv_enc", [E], F32, kind="ExternalInput")
    outT = nc.dram_tensor("outT", [E, Q], F32, kind="ExternalOutput")
    import os
    dbg = {}
    if os.environ.get("DEBUG_TAPS"):
        for nm, shp in [("d_xhat", [E, S]), ("d_xqh", [E, Q]), ("d_qT", [E, Q]),
                        ("d_kc0", [128, S]), ("d_vsb", [128, NKT * H * 65]),
                        ("d_aTat", [E, Q]), ("d_a1T", [E, Q]), ("d_aT", [E, Q]),
                        ("d_asum", [E, Q]), ("d_eaT", [E, Q]),
                        ("d_rsb", [128, 1024]), ("d_mursb", [128, 1024])]:
            dbg[nm] = nc.dram_tensor(nm, shp, F32, kind="ExternalOutput")


    wdr = {"qk": w_qk, "vs": w_vs, "proj": w_proj, "ma": w_ma, "q": w_q,
           "k": w_k, "v": w_v, "ep": w_ep, "a1": w_a1, "a2": w_a2,
           "fc": w_fc, "pj": w_pj}

    with tile.TileContext(nc) as tc:
        _emit(nc, tc, xT, xqT, encT, maskmul, mkT, mvA, wdr, bcols,
              bv_self, bv_enc, outT, dbg)
    nc.compile()
    return nc


def _emit(nc, tc, xT, xqT, encT, maskmul, mkT, mvA, wdr, bcols,
          bv_self, bv_enc, outT, dbg=None):
    dbg = dbg or {}

    def tap(name, ap):
        if name in dbg:
            nc.sync.dma_start(out=dbg[name][:, :].bitcast(ap.dtype), in_=ap)

    def tapc(name, chunks, T):
        if name in dbg:
            for c, ch in enumerate(chunks):
                nc.sync.dma_start(
                    out=dbg[name][c * 128:(c + 1) * 128, :].bitcast(ch.dtype),
                    in_=ch)

    from contextlib import ExitStack
    ctx = ExitStack()
    with ctx:
        dstp = ctx.enter_context(tc.tile_pool(name="dst", bufs=1, space="DRAM"))
        aTat_d = dstp.tile([E, Q], F32R, tag="aTat_d")
        a1T_d = dstp.tile([E, Q], F32R, tag="a1T_d")
        eaT_d = dstp.tile([E, Q], F32R, tag="eaT_d")
        e1p_d = dstp.tile([E, Q], F32, tag="e1p_d")

        def bounce_bcast(row_ap, out_ap, parts, n, dtag):
            """row [1, n] (SBUF) -> out [parts, n] via DRAM partition-broadcast."""
            d = dstp.tile([1, 1024], F32, tag=dtag, name="bounce", bufs=2)
            nc.sync.dma_start(out=d[:, 0:n], in_=row_ap)
            dv = d[0:1, 0:n]
            src_b = bass.AP(tensor=dv.tensor, offset=dv.offset,
                            ap=[[0, parts], list(dv.ap[-1])])
            nc.sync.dma_start(out=out_ap, in_=src_b)
        consts = ctx.enter_context(tc.tile_pool(name="consts", bufs=1))
        wslp = ctx.enter_context(tc.tile_pool(name="wsl", bufs=2))
        brdp = ctx.enter_context(tc.tile_pool(name="brd", bufs=1))
        tmpp = ctx.enter_context(tc.tile_pool(name="tmp", bufs=2))
        ptp = ctx.enter_context(tc.tile_pool(name="pt", bufs=2))
        rcpp = ctx.enter_context(tc.tile_pool(name="rcp", bufs=2))
        astp = ctx.enter_context(tc.tile_pool(name="ast", bufs=2))
        keyp = ctx.enter_context(tc.tile_pool(name="keyc", bufs=2))
        vsbp = ctx.enter_context(tc.tile_pool(name="vsbp", bufs=1))
        dsump = ctx.enter_context(tc.tile_pool(name="dsum", bufs=1))
        qqp = ctx.enter_context(tc.tile_pool(name="qq", bufs=1))
        n12p = ctx.enter_context(tc.tile_pool(name="n12", bufs=1))
        aTp = ctx.enter_context(tc.tile_pool(name="aTp", bufs=1))
        c24p = ctx.enter_context(tc.tile_pool(name="c24", bufs=1))
        psl = ctx.enter_context(tc.tile_pool(name="plin", bufs=2, space="PSUM"))
        pss = ctx.enter_context(tc.tile_pool(name="psc", bufs=1, space="PSUM"))
        psa = ctx.enter_context(tc.tile_pool(name="pav", bufs=2, space="PSUM"))

        # ---- constants ----
        ones_mm = consts.tile([128, 1], F32R)
        onesf = consts.tile([128, 12], F32)
        nc.vector.memset(onesf, 1.0)
        nc.vector.tensor_copy(out=ones_mm, in_=onesf[:, 0:1])
        bc = consts.tile([128, NB], F32)
        nc.sync.dma_start(out=bc, in_=bcols[:, :])
        mm_sb = consts.tile([128, NKT], F32)
        nc.sync.dma_start(out=mm_sb, in_=maskmul[:, :])
        mk_sb = consts.tile([128, KC, M], F32R)
        nc.sync.dma_start(out=mk_sb, in_=mkT[:, :, :])
        mv_sb = consts.tile([M, H * 65], F32R)
        nc.sync.dma_start(out=mv_sb, in_=mvA[:, :])
        bvb_s = consts.tile([128, E], F32)
        nc.sync.dma_start(out=bvb_s, in_=_row_bcast_dram(bv_self[:], 128))
        bvb_e = consts.tile([128, E], F32)
        nc.sync.dma_start(out=bvb_e, in_=_row_bcast_dram(bv_enc[:], 128))
        eps_t = consts.tile([128, 1], F32)
        nc.vector.memset(eps_t, EPS)

        def full_psv(ps):
            return bass.AP(tensor=ps.tensor, offset=ps.offset,
                           ap=[[ps.ap[0][0], 128]] + [list(a) for a in ps.ap[1:]])

        def stats_apply(srcs, T, out_chunks, dma_src=None):
            """Feature-major layernorm into out_chunks (list of KC APs [128,T]).

            srcs: KC source APs [128, T] (f32r) or None with dma_src(c) -> DRAM
            AP streamed twice. Stats vectors live in spare psum rows.
            """
            nt = T // 512

            def chunk(c):
                if srcs is not None:
                    return srcs[c]
                t = c24p.tile([128, 1024], F32R, tag="es", name="es", bufs=2)
                nc.sync.dma_start(out=t[:, 0:T], in_=dma_src(c))
                return t[:, 0:T]

            ps = psl.tile([128, 1024], F32, tag="lin", name="stps")
            psq = psl.tile([128, 1024], F32, tag="lin", name="stpq")
            for c in range(KC):
                src = chunk(c)
                sq = ptp.tile([128, 1024], F32R, tag="pt", name="sq")
                nc.vector.tensor_tensor(out=sq[:, 0:T], in0=src.bitcast(F32),
                                        in1=src.bitcast(F32), op=ALU.mult)
                for t in range(nt):
                    sl = slice(t * 512, (t + 1) * 512)
                    nc.tensor.matmul(ps[0:1, sl], ones_mm, src[:, sl],
                                     start=(c == 0), stop=(c == KC - 1))
                    nc.tensor.matmul(psq[0:1, sl], ones_mm, sq[:, sl],
                                     start=(c == 0), stop=(c == KC - 1))
            rs_b = brdp.tile([128, 1024], F32, tag="rsb")
            murs_b = brdp.tile([128, 1024], F32, tag="mursb")
            mu_s = murs_b[0:1, 0:T]
            rs_s = rs_b[0:1, 0:T]
            # mu -> sbuf; ex2 in-place in psum; then alternate psum/sbuf
            nc.vector.tensor_scalar(out=mu_s, in0=ps[0:1, 0:T],
                                    scalar1=1.0 / E, scalar2=None, op0=ALU.mult)
            nc.vector.tensor_scalar(out=psq[32:33, 0:T], in0=psq[0:1, 0:T],
                                    scalar1=1.0 / E, scalar2=None, op0=ALU.mult)
            nc.vector.tensor_tensor(out=rs_s, in0=mu_s, in1=mu_s, op=ALU.mult)
            nc.vector.tensor_tensor(out=psq[64:65, 0:T], in0=psq[32:33, 0:T],
                                    in1=rs_s, op=ALU.subtract)          # var
            nc.scalar.activation(out=psq[96:97, 0:T], in_=psq[64:65, 0:T],
                                 func=AF.Sqrt, bias=eps_t[0:1, :], scale=1.0)
            nc.vector.reciprocal(out=rs_s, in_=psq[96:97, 0:T])          # rs
            nc.vector.tensor_tensor(out=mu_s, in0=mu_s, in1=rs_s, op=ALU.mult)
            bounce_bcast(rs_b[0:1, 0:T], rs_b[1:128, 0:T], 127, T, "b_rs")
            bounce_bcast(murs_b[0:1, 0:T], murs_b[1:128, 0:T], 127, T, "b_mu")
            if T == Q and "d_rsb" in dbg and not dbg.get("_rsb_done"):
                dbg["_rsb_done"] = True
                nc.sync.dma_start(out=dbg["d_rsb"][:, :], in_=rs_b[:, :])
                nc.sync.dma_start(out=dbg["d_mursb"][:, :], in_=murs_b[:, :])
            for c in range(KC):
                src = chunk(c)
                o = out_chunks[c]
                nc.vector.tensor_tensor(out=o, in0=src.bitcast(F32),
                                        in1=rs_b[:, 0:T], op=ALU.mult)
                nc.vector.tensor_tensor(out=o, in0=o.bitcast(F32),
                                        in1=murs_b[:, 0:T], op=ALU.subtract)

        def wslice(wkey, nk, ft, kc0=0):
            w = wdr[wkey]
            t = wslp.tile([128, 12, 128], F32R, tag="wsl", name="wsl")
            src = w[:, :].rearrange("(c p) f -> p c f", p=128)
            src = src[:, kc0:kc0 + nk, ft * 128:(ft + 1) * 128]
            nc.sync.dma_start(out=t[:, 0:nk, :], in_=src)
            return t

        def linear_ps(rhs_fn, wkey, nk, ft, T, fcol0=0):
            nt = T // 512
            wt = wslice(wkey, nk, fcol0 + ft)
            ps = psl.tile([128, 1024], F32, tag="lin", name="linps")
            for kc in range(nk):
                for t in range(nt):
                    sl = slice(t * 512, (t + 1) * 512)
                    nc.tensor.matmul(ps[:, sl], wt[:, kc, :], rhs_fn(kc)[:, sl],
                                     start=(kc == 0), stop=(kc == nk - 1))
            return ps

        def linear(out_fn, rhs_fn, wkey, nk, nf, T, bias_col, act, fcol0=0):
            for ft in range(nf):
                ps = linear_ps(rhs_fn, wkey, nk, ft, T, fcol0)
                nc.scalar.activation(out=out_fn(ft), in_=ps[:, 0:T], func=act,
                                     bias=bc[:, bias_col + ft:bias_col + ft + 1],
                                     scale=1.0)

        def v_production(v_tile, src_fn, wv_half, h0, bias_b, masked):
            """Half the heads (h0=0 or 6): v cols [h0*64, h0*64+384)."""
            c0 = h0 * 64
            for tt in range(NKT):
                ps = psl.tile([128, 1024], F32, tag="lin", name="vps")
                for kc in range(KC):
                    nc.tensor.matmul(ps[:, 0:384],
                                     src_fn(kc)[:, tt * 128:(tt + 1) * 128],
                                     wv_half[:, kc, :],
                                     start=(kc == 0), stop=(kc == KC - 1))
                vrow = v_tile[:, tt, :].rearrange("p (h c) -> p h c", c=65)
                nc.vector.tensor_tensor(
                    out=vrow[:, h0:h0 + 6, 0:64],
                    in0=ps[:, 0:384].rearrange("p (h c) -> p h c", c=64),
                    in1=bias_b[:, c0:c0 + 384].rearrange("p (h c) -> p h c", c=64),
                    op=ALU.add)
                if masked:
                    nc.vector.tensor_scalar(
                        out=v_tile[:, tt, h0 * 65:(h0 + 6) * 65],
                        in0=v_tile[:, tt, h0 * 65:(h0 + 6) * 65].bitcast(F32),
                        scalar1=mm_sb[:, tt:tt + 1], scalar2=None, op0=ALU.mult)

        def make_v(v_tile, src_fn, wkey, bias_b, masked):
            for h0 in (0, 6):
                wvh = c24p.tile([128, KC, 384], F32R, tag="es", name="wvh", bufs=2)
                nc.sync.dma_start(
                    out=wvh,
                    in_=wdr[wkey][:, :].rearrange("(c p) f -> p c f", p=128)
                    [:, :, h0 * 64:h0 * 64 + 384])
                v_production(v_tile, src_fn, wvh, h0, bias_b, masked)

        def init_ones_cols(v_tile):
            for tt in range(NKT):
                vrow = v_tile[:, tt, :].rearrange("p (h c) -> p h c", c=65)
                nc.vector.tensor_copy(out=vrow[:, :, 64:65],
                                      in_=onesf[:, :].rearrange("p (h o) -> p h o", o=1))

        def attention(kchunk_fn, v_tile, q_tile, out_dram, scale, mem_dram=None):
            """q_tile [128, KC, Q]; out rows staged via SBUF -> out_dram [E, Q]."""
            for c in range(KC):
                kch = kchunk_fn(c)
                for hh in range(2):
                    h = 2 * c + hh
                    off = hh * 64
                    row0 = c * 128 + off
                    av = psa.tile([65, 512], F32, tag="av", name="av")
                    for half in range(NKT // 2):
                        sc = pss.tile([128, 1024], F32, tag="sc", name="sc")
                        for j in range(2):
                            kt = half * 2 + j
                            nc.tensor.matmul(
                                sc[:, j * 512:(j + 1) * 512],
                                kch[off:off + 64, kt * 128:(kt + 1) * 128],
                                q_tile[off:off + 64, c, :],
                                start=True, stop=True)
                        pt = ptp.tile([128, 1024], F32R, tag="pt", name="pt")
                        nc.scalar.activation(out=pt, in_=sc[:, :], func=AF.Exp,
                                             scale=scale)
                        for j in range(2):
                            kt = half * 2 + j
                            nc.tensor.matmul(
                                av[:, :], v_tile[:, kt, h * 65:(h + 1) * 65],
                                pt[:, j * 512:(j + 1) * 512],
                                start=(kt == 0), stop=(kt == NKT - 1))
                    rcp = rcpp.tile([1, 512], F32, tag="rcp", name="rcp")
                    nc.vector.reciprocal(out=rcp, in_=av[64:65, :])
                    rcb = rcpp.tile([64, 512], F32, tag="rcb", name="rcb")
                    bounce_bcast(rcp[:, :], rcb[:, :], 64, 512, "b_rc")
                    st = astp.tile([64, 512], F32R, tag="ast", name="ast")
                    nc.vector.tensor_tensor(out=st, in0=av[0:64, :], in1=rcb,
                                            op=ALU.mult)
                    nc.sync.dma_start(out=out_dram[row0:row0 + 64, :], in_=st)
                    if mem_dram is not None:
                        scm = pss.tile([128, 1024], F32, tag="sc", name="scm")
                        nc.tensor.matmul(scm[0:M, 0:512], mk_sb[off:off + 64, c, :],
                                         q_tile[off:off + 64, c, :],
                                         start=True, stop=True)
                        pmt = ptp.tile([128, 1024], F32R, tag="pt", name="pmt")
                        nc.scalar.activation(out=pmt[0:M, 0:512], in_=scm[0:M, 0:512],
                                             func=AF.Exp, scale=1.0)
                        av1 = psa.tile([65, 512], F32, tag="av", name="av1")
                        nc.tensor.matmul(av1[:, :], mv_sb[:, h * 65:(h + 1) * 65],
                                         pmt[0:M, 0:512], start=True, stop=True)
                        rcp1 = rcpp.tile([1, 512], F32, tag="rcp", name="rcp1")
                        nc.vector.reciprocal(out=rcp1, in_=av1[64:65, :])
                        rcb1 = rcpp.tile([64, 512], F32, tag="rcb", name="rcb1")
                        bounce_bcast(rcp1[:, :], rcb1[:, :], 64, 512, "b_rc")
                        st1 = astp.tile([64, 512], F32R, tag="ast", name="ast1")
                        nc.vector.tensor_tensor(out=st1, in0=av1[0:64, :], in1=rcb1,
                                                op=ALU.mult)
                        nc.sync.dma_start(out=mem_dram[row0:row0 + 64, :], in_=st1)

        # ---- persistent tiles ----
        v_sb = vsbp.tile([128, NKT, H * 65], F32R, tag="vsb")
        init_ones_cols(v_sb)
        asum = dsump.tile([128, KC, Q], F32R, tag="asum")

        # ======== phase A: xhat (streamed), xqhat ========
        xhat = c24p.tile([128, KC, 1024], F32R, tag="c24", name="xhat")
        stats_apply(None, S, [xhat[:, c, :] for c in range(KC)],
                    dma_src=lambda c: xT[c * 128:(c + 1) * 128, :])
        xqh = n12p.tile([128, KC, Q], F32R, tag="n12", name="xqh")
        stats_apply(None, Q, [xqh[:, c, :] for c in range(KC)],
                    dma_src=lambda c: xqT[c * 128:(c + 1) * 128, :])
        tapc("d_xhat", [xhat[:, c, :] for c in range(KC)], S)
        tapc("d_xqh", [xqh[:, c, :] for c in range(KC)], Q)

        # ======== phase B: v, q, then self+memory attention ========
        make_v(v_sb, lambda kc: xhat[:, kc, :], "vs", bvb_s, masked=False)
        tap("d_vsb", v_sb[:, :, :])
        qT = qqp.tile([128, KC, Q], F32R, tag="qq", name="qT")
        linear(lambda ft: qT[:, ft, :], lambda kc: xqh[:, kc, :],
               "qk", KC, KC, Q, BC_Q, AF.Identity)
        tapc("d_qT", [qT[:, c, :] for c in range(KC)], Q)

        def self_kchunk(c):
            kt = keyp.tile([128, S], F32R, tag="keyc", name="kch")
            ps = linear_ps(lambda kc: xhat[:, kc, :], "qk", KC, c, S, fcol0=KC)
            nc.scalar.activation(out=kt, in_=ps[:, :], func=AF.Identity,
                                 bias=bc[:, BC_K + c:BC_K + c + 1], scale=1.0)
            if c == 0:
                tap("d_kc0", kt[:, :])
            return kt

        attention(self_kchunk, v_sb, qT, aTat_d, 1.0, mem_dram=a1T_d)
        tap("d_aTat", aTat_d[:, :])
        tap("d_a1T", a1T_d[:, :])

        # ======== phase B4: gate + attn_proj + residual ========
        aT = aTp.tile([128, KC, Q], F32R, tag="aTp", name="aT")
        gAB = c24p.tile([128, 12, Q], F32R, tag="c24", name="gAB")
        nc.sync.dma_start(out=gAB[:, 0:KC, :],
                          in_=aTat_d[:, :].rearrange("(c p) t -> p c t", p=128))
        nc.sync.dma_start(out=gAB[:, KC:12, :],
                          in_=a1T_d[:, :].rearrange("(c p) t -> p c t", p=128))
        aN = n12p.tile([128, KC, Q], F32R, tag="n12", name="aN")
        for ft in range(KC):
            ps = linear_ps(lambda kc: gAB[:, kc, :], "ma", 12, ft, Q)
            al = tmpp.tile([128, 1024], F32, tag="al", name="al")
            nc.scalar.activation(out=al[:, 0:Q], in_=ps[:, 0:Q], func=AF.Sigmoid,
                                 bias=bc[:, BC_MA + ft:BC_MA + ft + 1], scale=1.0)
            d = tmpp.tile([128, 1024], F32, tag="nrm", name="d")
            nc.vector.tensor_tensor(out=d[:, 0:Q], in0=gAB[:, ft, :].bitcast(F32),
                                    in1=gAB[:, KC + ft, :].bitcast(F32),
                                    op=ALU.subtract)
            nc.vector.tensor_tensor(out=d[:, 0:Q], in0=al[:, 0:Q], in1=d[:, 0:Q],
                                    op=ALU.mult)
            nc.vector.tensor_tensor(out=aN[:, ft, :],
                                    in0=gAB[:, KC + ft, :].bitcast(F32),
                                    in1=d[:, 0:Q], op=ALU.add)
        for ft in range(KC):
            ps = linear_ps(lambda kc: aN[:, kc, :], "proj", KC, ft, Q)
            xq_c = c24p.tile([128, 1024], F32R, tag="es", name="xqc", bufs=2)
            nc.sync.dma_start(out=xq_c[:, 0:Q],
                              in_=xqT[ft * 128:(ft + 1) * 128, :])
            nc.vector.scalar_tensor_tensor(
                out=aT[:, ft, :], in0=ps[:, 0:Q],
                scalar=bc[:, BC_PROJ + ft:BC_PROJ + ft + 1],
                in1=xq_c[:, 0:Q].bitcast(F32), op0=ALU.add, op1=ALU.add)

        # ======== phase C: hahat + qe ========
        tapc("d_aT", [aT[:, c, :] for c in range(KC)], Q)
        hah = n12p.tile([128, KC, Q], F32R, tag="n12", name="hah")
        stats_apply([aT[:, c, :] for c in range(KC)], Q,
                    [hah[:, c, :] for c in range(KC)])
        qeT = qqp.tile([128, KC, Q], F32R, tag="qq", name="qeT")
        linear(lambda ft: qeT[:, ft, :], lambda kc: hah[:, kc, :],
               "q", KC, KC, Q, BC_FCQ, AF.Identity)

        # ======== phase D: two cross-attentions ========
        for e in range(2):
            ehat = c24p.tile([128, KC, 1024], F32R, tag="c24", name="ehat")
            stats_apply(None, SE, [ehat[:, c, :] for c in range(KC)],
                        dma_src=lambda c, _e=e: encT[_e, c * 128:(c + 1) * 128, :])
            make_v(v_sb, lambda kc: ehat[:, kc, :], "v", bvb_e, masked=True)

            def enc_kchunk(c, _ehat=ehat):
                kt = keyp.tile([128, S], F32R, tag="keyc", name="kche")
                ps = linear_ps(lambda kc: _ehat[:, kc, :], "k", KC, c, SE)
                nc.scalar.activation(out=kt, in_=ps[:, :], func=AF.Identity,
                                     bias=bc[:, BC_FCK + c:BC_FCK + c + 1],
                                     scale=1.0)
                return kt

            attention(enc_kchunk, v_sb, qeT, eaT_d, 0.125)
            if e == 0:
                tap("d_eaT", eaT_d[:, :])

            # enc_proj from staged eaT, alpha gate, combine
            ee = c24p.tile([128, 12, Q], F32R, tag="c24", name="ee")
            nc.sync.dma_start(out=ee[:, 0:KC, :],
                              in_=eaT_d[:, :].rearrange("(c p) t -> p c t", p=128))
            linear(lambda ft: ee[:, KC + ft, :], lambda kc: ee[:, kc, :],
                   "ep", KC, KC, Q, BC_EP, AF.Identity)
            bcol0 = BC_A1 if e == 0 else BC_A2

            def alpha_rhs(kc):
                return aT[:, kc, :] if kc < KC else ee[:, KC + kc - KC, :]

            for ft in range(KC):
                ps = linear_ps(alpha_rhs, "a1" if e == 0 else "a2", 12, ft, Q)
                al = tmpp.tile([128, 1024], F32, tag="al", name="alE")
                nc.scalar.activation(out=al[:, 0:Q], in_=ps[:, 0:Q], func=AF.Sigmoid,
                                     bias=bc[:, bcol0 + ft:bcol0 + ft + 1], scale=1.0)
                d = tmpp.tile([128, 1024], F32, tag="nrm", name="dE")
                nc.vector.tensor_tensor(out=d[:, 0:Q], in0=aT[:, ft, :].bitcast(F32),
                                        in1=ee[:, KC + ft, :].bitcast(F32),
                                        op=ALU.subtract)
                nc.vector.tensor_tensor(out=d[:, 0:Q], in0=al[:, 0:Q], in1=d[:, 0:Q],
                                        op=ALU.mult)
                nc.vector.tensor_tensor(out=d[:, 0:Q],
                                        in0=ee[:, KC + ft, :].bitcast(F32),
                                        in1=d[:, 0:Q], op=ALU.add)
                if e == 0:
                    nc.sync.dma_start(out=e1p_d[ft * 128:(ft + 1) * 128, :],
                                      in_=d[:, 0:Q])
                else:
                    ep1 = tmpp.tile([128, 1024], F32, tag="al", name="e1b")
                    nc.sync.dma_start(out=ep1[:, 0:Q],
                                      in_=e1p_d[ft * 128:(ft + 1) * 128, :])
                    nc.vector.tensor_tensor(out=asum[:, ft, :], in0=ep1[:, 0:Q],
                                            in1=d[:, 0:Q], op=ALU.add)

        # ======== phase E/F: MLP + final residual ========
        tapc("d_asum", [asum[:, c, :] for c in range(KC)], Q)
        hm2 = n12p.tile([128, KC, Q], F32R, tag="n12", name="hm2")
        stats_apply([asum[:, c, :] for c in range(KC)], Q,
                    [hm2[:, c, :] for c in range(KC)])
        mstage = aTp.tile([128, KC, Q], F32, tag="aTp", name="mstage")
        for mh in range(2):
            mT = c24p.tile([128, 12, Q], F32R, tag="c24", name="mT")
            linear(lambda ft: mT[:, ft, :], lambda kc: hm2[:, kc, :],
                   "fc", KC, 12, Q, BC_FC + 12 * mh, AF.Gelu_apprx_tanh,
                   fcol0=12 * mh)
            for ft in range(KC):
                wt = wslice("pj", 12, ft, kc0=12 * mh)
                ps = psl.tile([128, 1024], F32, tag="lin", name="pjps")
                for kc in range(12):
                    nc.tensor.matmul(ps[:, 0:512], wt[:, kc, :], mT[:, kc, :],
                                     start=(kc == 0), stop=(kc == 11))
                if mh == 0:
                    nc.scalar.activation(out=mstage[:, ft, :], in_=ps[:, 0:Q],
                                         func=AF.Identity,
                                         bias=bc[:, BC_PJ + ft:BC_PJ + ft + 1],
                                         scale=1.0)
                else:
                    t = tmpp.tile([128, 1024], F32, tag="nrm", name="mo")
                    nc.vector.scalar_tensor_tensor(
                        out=t[:, 0:Q], in0=asum[:, ft, :].bitcast(F32),
                        scalar=float(1.0 / np.sqrt(2.0)), in1=ps[:, 0:Q],
                        op0=ALU.mult, op1=ALU.add)
                    ot = tmpp.tile([128, 1024], F32, tag="al", name="ot")
                    nc.vector.tensor_tensor(out=ot[:, 0:Q], in0=t[:, 0:Q],
                                            in1=mstage[:, ft, :], op=ALU.add)
                    nc.sync.dma_start(out=outT[ft * 128:(ft + 1) * 128, :],
                                      in_=ot[:, 0:Q])


_NC_CACHE = None


def _get_nc():
    global _NC_CACHE
    if _NC_CACHE is None:
        _NC_CACHE = build_program()
    return _NC_CACHE


def _pack_bias_cols(seg_biases):
    bcols = np.zeros((128, NB), np.float32)
    for col0, b in seg_biases:
        nf = b.shape[0] // 128
        bcols[:, col0:col0 + nf] = b.reshape(nf, 128).T
    return bcols


def kernel(x, encoder_features, mask_encoder, ln1_g, ln1_b, ln2_g, ln2_b,
           c_attn_w, c_attn_b, attn_proj_w, attn_proj_b,
           memory_features, mem_attn_w, mem_attn_b, mem_alpha_w, mem_alpha_b,
           fcq_w, fcq_b, fck_w, fck_b, fcv_w, fcv_b, enc_proj_w, enc_proj_b,
           fc_alpha1_w, fc_alpha1_b, fc_alpha2_w, fc_alpha2_b,
           mlp_fc_w, mlp_fc_b, mlp_proj_w, mlp_proj_b):
    f32 = np.float32
    x = np.asarray(x, f32)
    encoder_features = np.asarray(encoder_features, f32)

    # ---- fold LN gains/biases into consumer weights ----
    g1 = np.asarray(ln1_g, f32); b1 = np.asarray(ln1_b, f32)
    g2 = np.asarray(ln2_g, f32); b2 = np.asarray(ln2_b, f32)

    def fold(w, b, g, lb):
        w = np.asarray(w, f32); b = np.asarray(b, f32)
        return (w * g[:, None]).astype(f32), (lb @ w + b).astype(f32)

    w_qkv, b_qkv = fold(c_attn_w, c_attn_b, g1, b1)
    w_fcq, b_fcq = fold(fcq_w, fcq_b, g1, b1)
    w_fck, b_fck = fold(fck_w, fck_b, g1, b1)
    w_fcv, b_fcv = fold(fcv_w, fcv_b, g1, b1)
    w_mfc, b_mfc = fold(mlp_fc_w, mlp_fc_b, g2, b2)

    # ---- memory slots (batch independent) ----
    mem = (np.asarray(memory_features, f32)[0] @ np.asarray(mem_attn_w, f32)
           + np.asarray(mem_attn_b, f32))          # [M, 2E]
    mk = mem[:, :E].reshape(M, H, D)
    mv = mem[:, E:].reshape(M, H, D)
    mkT = np.zeros((128, KC, M), f32)
    mvA = np.zeros((M, H * 65), f32)
    for h in range(H):
        c, off = divmod(h, 2)
        mkT[off * 64:(off + 1) * 64, c, :] = mk[:, h, :].T
        mvA[:, h * 65:h * 65 + 64] = mv[:, h, :]
        mvA[:, h * 65 + 64] = 1.0

    bcols = _pack_bias_cols([
        (BC_Q, b_qkv[0:E]), (BC_K, b_qkv[E:2 * E]),
        (BC_PROJ, np.asarray(attn_proj_b, f32)),
        (BC_MA, np.asarray(mem_alpha_b, f32)),
        (BC_FCQ, b_fcq), (BC_FCK, b_fck),
        (BC_EP, np.asarray(enc_proj_b, f32)),
        (BC_A1, np.asarray(fc_alpha1_b, f32)),
        (BC_A2, np.asarray(fc_alpha2_b, f32)),
        (BC_FC, b_mfc), (BC_PJ, np.asarray(mlp_proj_b, f32)),
    ])

    keep = (~np.asarray(mask_encoder, bool)[:, 0, 0, :]).astype(f32)  # [B, SE]

    common = dict(
        mkT=mkT, mvA=mvA,
        w_qk=np.ascontiguousarray(w_qkv[:, 0:2 * E]),
        w_vs=np.ascontiguousarray(w_qkv[:, 2 * E:3 * E]),
        w_proj=np.asarray(attn_proj_w, f32),
        w_ma=np.asarray(mem_alpha_w, f32),
        w_q=w_fcq, w_k=w_fck, w_v=w_fcv,
        w_ep=np.asarray(enc_proj_w, f32),
        w_a1=np.asarray(fc_alpha1_w, f32),
        w_a2=np.asarray(fc_alpha2_w, f32),
        w_fc=w_mfc, w_pj=np.asarray(mlp_proj_w, f32),
        bcols=bcols,
        bv_self=np.ascontiguousarray(b_qkv[2 * E:3 * E]),
        bv_enc=b_fcv,
    )

    in_maps = []
    for core in range(8):
        b, half = divmod(core, 2)
        xTb = np.ascontiguousarray(x[b].T)                       # [E, S]
        m = dict(common)
        m["xT"] = xTb
        m["xqT"] = np.ascontiguousarray(xTb[:, half * Q:(half + 1) * Q])
        m["encT"] = np.ascontiguousarray(encoder_features[b].transpose(0, 2, 1))
        m["maskmul"] = np.ascontiguousarray(keep[b].reshape(NKT, 128).T)
        in_maps.append(m)

    nc = _get_nc()
    res = run_bass_kernel_spmd(nc, in_maps, core_ids=list(range(8)))

    global _LAST_IN_MAPS
    _LAST_IN_MAPS = in_maps

    y = np.empty((B, S, E), f32)
    for core in range(8):
        b, half = divmod(core, 2)
        y[b, half * Q:(half + 1) * Q, :] = res.results[core]["outT"].T
    return y


_LAST_IN_MAPS = None


def profile_exec_ns(n_hot=12, n_cold=2):
    """Estimate per-invocation device time by timing pipelined repeats of the
    jitted 8-core executable with device-resident inputs."""
    import time
    import jax
    from jax.sharding import Mesh, PartitionSpec
    from jax.experimental.shard_map import shard_map
    import concourse.mybir as mybir_
    from concourse import bass2jax

    if _LAST_IN_MAPS is None:
        return None
    nc = _get_nc()
    in_maps = _LAST_IN_MAPS
    n_cores = 8
    bass2jax.install_neuronx_cc_hook()

    in_names, out_names, out_avals, zero_outs = [], [], [], []
    partition_name = nc.partition_id_tensor.name if nc.partition_id_tensor else None
    for alloc in nc.m.functions[0].allocations:
        if not isinstance(alloc, mybir_.MemoryLocationSet):
            continue
        name = alloc.memorylocations[0].name
        if alloc.kind == "ExternalInput":
            if name != partition_name:
                in_names.append(name)
        elif alloc.kind == "ExternalOutput":
            out_avals.append(jax.core.ShapedArray(
                tuple(alloc.tensor_shape), mybir_.dt.np(alloc.dtype)))
            zero_outs.append(np.zeros(tuple(alloc.tensor_shape),
                                      mybir_.dt.np(alloc.dtype)))
            out_names.append(name)
    n_params = len(in_names)
    n_outs = len(out_avals)
    all_in_names = in_names + out_names + ([partition_name] if partition_name else [])
    donate = tuple(range(n_params, n_params + n_outs))

    def _body(*args):
        operands = list(args)
        if partition_name is not None:
            operands.append(bass2jax.partition_id_tensor())
        return tuple(bass2jax._bass_exec_p.bind(
            *operands, out_avals=tuple(out_avals), in_names=tuple(all_in_names),
            out_names=tuple(out_names), lowering_input_output_aliases=(),
            sim_require_finite=True, sim_require_nnan=True, nc=nc))

    devices = jax.devices()[:n_cores]
    mesh = Mesh(np.asarray(devices), ("core",))
    fn = jax.jit(shard_map(_body, mesh=mesh,
                           in_specs=(PartitionSpec("core"),) * (n_params + n_outs),
                           out_specs=(PartitionSpec("core"),) * n_outs,
                           check_rep=False),
                 donate_argnums=donate, keep_unused=True)
    sh = jax.sharding.NamedSharding(mesh, PartitionSpec("core"))
    concat_in = [jax.device_put(
        np.concatenate([np.asarray(in_maps[c][nm]) for c in range(n_cores)], 0), sh)
        for nm in in_names]

    def zeros():
        return [jax.device_put(
            np.zeros((n_cores * z.shape[0], *z.shape[1:]), z.dtype), sh)
            for z in zero_outs]

    def run(n):
        o = tuple(zeros())
        o = fn(*concat_in, *o)
        jax.block_until_ready(o)
        t0 = time.perf_counter()
        for _ in range(n):
            o = fn(*concat_in, *o)
        jax.block_until_ready(o)
        return time.perf_counter() - t0

    tc = run(n_cold)
    th = run(n_hot)
    per = (th - tc) / (n_hot - n_cold)
    print(f"pipelined wall: {n_cold} calls {tc*1e3:.2f} ms, "
          f"{n_hot} calls {th*1e3:.2f} ms -> per-call {per*1e6:.0f} us")
    return int(per * 1e9)



# revision 12
# speedup vs baseline: 2.0523x; 2.0523x over previous
"""Trainium2 Bass kernel for nn_Block_23338852286694 (dense transformer block), v2.

Sharding: 8 cores = 4 batches x 2 query-halves. Each core computes the full
block for its 512 query tokens (K/V work over the full 1024-token sequence is
duplicated across the pair of cores sharing a batch; no collectives).

v2 changes vs baseline:
- LayerNorm: single-pass (DMA once into the target tile, stats, in-place
  normalize); rs/mu*rs rows broadcast across partitions via a ones-row PE
  matmul into spare PSUM banks (no DRAM bounce). xq LN is a column slice of
  the full-x LN (it was recomputed from scratch before).
- Attention epilogue: softmax denominators reciprocal'd on DVE, broadcast via
  gpsimd.partition_broadcast (Pool engine) into SBUF, and the normalized
  output written directly into a feature-major SBUF tile using
  partition-shifted DVE ops (hh=1 heads write partitions 64:128). No DRAM
  staging of attention outputs anywhere.
- All weights pre-packed on the host into [128, NF, KC, 128] so every weight
  slice DMA is a clean 2D transfer with 3KB contiguous per partition.
- PSUM: psl [128,512]x4 for linears/stats, psc [128,512]x2 for score tiles,
  psa [128,512]x2 for AV accumulators; score row-tiles of a head pair run
  concurrently in the PE array (K=64 row tiling via base partitions).
- Elementwise squares for LN stats and half the normalize/gating elementwise
  work run on the otherwise-idle Pool (gpsimd) engine.
- LN stats accumulate in the attention PSUM pools so the linear-phase PSUM
  rotation never stalls behind the serial LN row chain.
- The MLP (fc/pj) runs fully in bf16 (weights, LN output, gelu output):
  same PE rate, half the weight DMA traffic.
"""
import numpy as np

import concourse.bass as bass
import concourse.bacc as bacc
import concourse.mybir as mybir
import concourse.tile as tile
from concourse.bass_utils import run_bass_kernel_spmd

F32 = mybir.dt.float32
F32R = mybir.dt.float32r
BF16 = mybir.dt.bfloat16
AF = mybir.ActivationFunctionType
ALU = mybir.AluOpType

B, S, SE = 4, 1024, 1024
E, H, M, D = 768, 12, 100, 64
KC = E // 128            # 6 feature chunks
Q = S // 2               # 512 query tokens per core
EPS = 1e-5
NKT = S // 128           # 8 key tiles

# packed per-partition bias column offsets (see _pack_bias_cols)
BC_Q, BC_K, BC_PROJ, BC_MA, BC_FCQ, BC_FCK, BC_EP, BC_A1, BC_A2, BC_FC, BC_PJ = (
    0, 6, 12, 18, 24, 30, 36, 42, 48, 54, 78)
NB = 84


def build_program():
    nc = bacc.Bacc(trn_type="TRN2")

    # ---- inputs (per-core; host pre-transposes / pre-folds / pre-packs) ----
    xT = nc.dram_tensor("xT", [E, S], F32R, kind="ExternalInput")
    xqT = nc.dram_tensor("xqT", [E, Q], F32R, kind="ExternalInput")
    encT = nc.dram_tensor("encT", [2, E, SE], F32R, kind="ExternalInput")
    maskmul = nc.dram_tensor("maskmul", [128, NKT], F32, kind="ExternalInput")
    mkT = nc.dram_tensor("mkT", [128, KC, M], F32R, kind="ExternalInput")
    mvA = nc.dram_tensor("mvA", [M, H * 65], F32R, kind="ExternalInput")
    # packed weights [128, NF, KCW, 128]
    wp_qk = nc.dram_tensor("wp_qk", [128, 12, KC, 128], F32R, kind="ExternalInput")
    wp_proj = nc.dram_tensor("wp_proj", [128, KC, KC, 128], F32R, kind="ExternalInput")
    wp_ma = nc.dram_tensor("wp_ma", [128, KC, 12, 128], F32R, kind="ExternalInput")
    wp_q = nc.dram_tensor("wp_q", [128, KC, KC, 128], F32R, kind="ExternalInput")
    wp_k = nc.dram_tensor("wp_k", [128, KC, KC, 128], F32R, kind="ExternalInput")
    wp_ep = nc.dram_tensor("wp_ep", [128, KC, KC, 128], F32R, kind="ExternalInput")
    wp_a1 = nc.dram_tensor("wp_a1", [128, KC, 12, 128], F32R, kind="ExternalInput")
    wp_a2 = nc.dram_tensor("wp_a2", [128, KC, 12, 128], F32R, kind="ExternalInput")
    wp_fc = nc.dram_tensor("wp_fc", [128, 24, KC, 128], BF16, kind="ExternalInput")
    wp_pj = nc.dram_tensor("wp_pj", [128, KC, 24, 128], BF16, kind="ExternalInput")
    # v-production weights [128, half, KCW, 384]
    wv_self = nc.dram_tensor("wv_self", [128, 2, KC, 384], F32R, kind="ExternalInput")
    wv_enc = nc.dram_tensor("wv_enc", [128, 2, KC, 384], F32R, kind="ExternalInput")
    bcols = nc.dram_tensor("bcols", [128, NB], F32, kind="ExternalInput")
    bv_self = nc.dram_tensor("bv_self", [E], F32, kind="ExternalInput")
    bv_enc = nc.dram_tensor("bv_enc", [E], F32, kind="ExternalInput")
    outT = nc.dram_tensor("outT", [E, Q], F32, kind="ExternalOutput")

    import os
    dbg = {}
    if os.environ.get("DEBUG_TAPS"):
        for nm, shp in [("d_xhat", [E, S]), ("d_qT", [E, Q]), ("d_aat", [E, Q]),
                        ("d_amem", [E, Q]), ("d_aT", [E, Q]), ("d_asum", [E, Q]),
                        ("d_hah", [E, Q]), ("d_eaT", [E, Q])]:
            dbg[nm] = nc.dram_tensor(nm, shp, F32, kind="ExternalOutput")

    wdr = {"qk": wp_qk, "proj": wp_proj, "ma": wp_ma, "q": wp_q, "k": wp_k,
           "ep": wp_ep, "a1": wp_a1, "a2": wp_a2, "fc": wp_fc, "pj": wp_pj}

    with tile.TileContext(nc) as tc:
        _emit(nc, tc, xT, xqT, encT, maskmul, mkT, mvA, wdr, wv_self, wv_enc,
              bcols, bv_self, bv_enc, outT, dbg)
    nc.compile()
    return nc


def _emit(nc, tc, xT, xqT, encT, maskmul, mkT, mvA, wdr, wv_self, wv_enc,
          bcols, bv_self, bv_enc, outT, dbg=None):
    dbg = dbg or {}

    def tapc(name, chunks):
        if name in dbg:
            for c, ch in enumerate(chunks):
                nc.sync.dma_start(
                    out=dbg[name][c * 128:(c + 1) * 128, :].bitcast(ch.dtype),
                    in_=ch)

    from contextlib import ExitStack
    ctx = ExitStack()
    with ctx:
        consts = ctx.enter_context(tc.tile_pool(name="consts", bufs=1))
        bigp = ctx.enter_context(tc.tile_pool(name="big", bufs=2))
        n12p = ctx.enter_context(tc.tile_pool(name="n12", bufs=1))
        vsbp = ctx.enter_context(tc.tile_pool(name="vsb", bufs=1))
        keyp = ctx.enter_context(tc.tile_pool(name="keyc", bufs=2))
        wslp = ctx.enter_context(tc.tile_pool(name="wsl", bufs=2))
        wvp = ctx.enter_context(tc.tile_pool(name="wvp", bufs=1))
        ptp = ctx.enter_context(tc.tile_pool(name="pt", bufs=2))
        rcbp = ctx.enter_context(tc.tile_pool(name="rcb", bufs=2))
        rowp = ctx.enter_context(tc.tile_pool(name="row", bufs=4))
        tmpp = ctx.enter_context(tc.tile_pool(name="tmp", bufs=2))
        psl = ctx.enter_context(tc.tile_pool(name="plin", bufs=4, space="PSUM"))
        psc = ctx.enter_context(tc.tile_pool(name="psc", bufs=2, space="PSUM"))
        psa = ctx.enter_context(tc.tile_pool(name="pav", bufs=2, space="PSUM"))

        # ---- input stream first (priority: feeds everything) ----
        xhat = bigp.tile([128, KC, 1024], F32R, tag="big", name="xhat")
        for _c in range(KC):
            nc.sync.dma_start(out=xhat[:, _c, :],
                              in_=xT[_c * 128:(_c + 1) * 128, :])

        # ---- constants ----
        onesf = consts.tile([128, 12], F32)
        nc.vector.memset(onesf, 1.0)
        ones_mm = consts.tile([128, 1], F32R)
        nc.vector.tensor_copy(out=ones_mm, in_=onesf[:, 0:1])
        ones_rf = consts.tile([1, 128], F32)
        nc.vector.memset(ones_rf, 1.0)
        ones_row = consts.tile([1, 128], F32R)
        nc.vector.tensor_copy(out=ones_row, in_=ones_rf)
        bc = consts.tile([128, NB], F32)
        nc.sync.dma_start(out=bc, in_=bcols[:, :])
        mm_sb = consts.tile([128, NKT], F32)
        nc.sync.dma_start(out=mm_sb, in_=maskmul[:, :])
        mk_sb = consts.tile([128, KC, M], F32R)
        nc.sync.dma_start(out=mk_sb, in_=mkT[:, :, :])
        mv_sb = consts.tile([M, H * 65], F32R)
        nc.sync.dma_start(out=mv_sb, in_=mvA[:, :])
        bvb_s = consts.tile([128, E], F32)
        nc.sync.dma_start(out=bvb_s, in_=bass.AP(
            tensor=bv_self[:].tensor, offset=bv_self[:].offset,
            ap=[[0, 128], list(bv_self[:].ap[-1])]))
        bvb_e = consts.tile([128, E], F32)
        nc.sync.dma_start(out=bvb_e, in_=bass.AP(
            tensor=bv_enc[:].tensor, offset=bv_enc[:].offset,
            ap=[[0, 128], list(bv_enc[:].ap[-1])]))
        eps_t = consts.tile([128, 1], F32)
        nc.vector.memset(eps_t, EPS)

        # ---------------- helpers ----------------
        def wslice(wkey, nk, ft, kc0=0, nf=1, dt=F32R):
            wt = wslp.tile([128, 12, 128], dt, tag="wsl", name="wsl")
            w4 = wt.rearrange("p (f c) x -> p f c x", c=nk) if nf > 1 else None
            if nf > 1:
                nc.sync.dma_start(out=w4[:, 0:nf, :, :],
                                  in_=wdr[wkey][:, ft:ft + nf, kc0:kc0 + nk, :])
            else:
                nc.sync.dma_start(out=wt[:, 0:nk, :],
                                  in_=wdr[wkey][:, ft, kc0:kc0 + nk, :])
            return wt

        wcache = {}

        def linear_ps(rhs_fn, wkey, nk, ft, T, fcol0=0):
            """Return list of [128,512] psum tiles covering T columns.

            For nk==6 weights, slices are fetched two output-tiles at a time
            (6 KB contiguous per partition) and cached for the odd ft."""
            nt = T // 512
            wdt = BF16 if wkey in ("fc", "pj") else F32R
            if nk == 6:
                key = (wkey, fcol0 + ft - (ft % 2))
                ent = wcache.get(key)
                if ent is None:
                    wt2 = wslice(wkey, nk, fcol0 + ft - (ft % 2), nf=2, dt=wdt)
                    wcache.clear()
                    wcache[key] = wt2
                else:
                    wt2 = ent
                wt = wt2.rearrange("p (f c) x -> p f c x", c=nk)[:, ft % 2]
            else:
                wt = wslice(wkey, nk, fcol0 + ft, dt=wdt)
            tiles = []
            for t in range(nt):
                ps = psl.tile([128, 512], F32, tag="lin", name="linps")
                tiles.append(ps)
            for kc in range(nk):
                for t in range(nt):
                    sl = slice(t * 512, (t + 1) * 512)
                    nc.tensor.matmul(tiles[t][:, :], wt[:, kc, :],
                                     rhs_fn(kc)[:, sl],
                                     start=(kc == 0), stop=(kc == nk - 1))
            return tiles

        def linear(out_fn, rhs_fn, wkey, nk, nf, T, bias_col, act, fcol0=0):
            nt = T // 512
            for ft in range(nf):
                tiles = linear_ps(rhs_fn, wkey, nk, ft, T, fcol0)
                for t in range(nt):
                    nc.scalar.activation(
                        out=out_fn(ft)[:, t * 512:(t + 1) * 512],
                        in_=tiles[t][:, :], func=act,
                        bias=bc[:, bias_col + ft:bias_col + ft + 1], scale=1.0)

        def ln_dma(tile_chunks, dma_src):
            for c in range(KC):
                nc.sync.dma_start(out=tile_chunks[c], in_=dma_src(c))

        def stats_norm(tile_chunks, T, out_chunks, dma_src=None):
            """LN over feature chunks. tile_chunks: list of KC [128,T] APs
            (f32r). If dma_src given, first DMA dma_src(c) into tile_chunks[c].
            Normalized result written to out_chunks[c] (may alias input)."""
            nt = T // 512
            if dma_src is not None:
                ln_dma(tile_chunks, dma_src)
            ps_s = [psc.tile([128, 512], F32, tag="sc", name="stps")
                    for _ in range(nt)]
            ps_q = [psa.tile([128, 512], F32, tag="av", name="stpq")
                    for _ in range(nt)]
            for c in range(KC):
                src = tile_chunks[c]
                for t in range(nt):
                    sl = slice(t * 512, (t + 1) * 512)
                    sq = ptp.tile([128, 512], F32R, tag="pt", name="sq")
                    nc.gpsimd.tensor_tensor(out=sq, in0=src[:, sl].bitcast(F32),
                                            in1=src[:, sl].bitcast(F32),
                                            op=ALU.mult)
                    nc.tensor.matmul(ps_s[t][0:1, :], ones_mm, src[:, sl],
                                     start=(c == 0), stop=(c == KC - 1))
                    nc.tensor.matmul(ps_q[t][0:1, :], ones_mm, sq,
                                     start=(c == 0), stop=(c == KC - 1))
            rs_bc, mu_bc = [], []
            for t in range(nt):
                s, q = ps_s[t], ps_q[t]
                # mu (sbuf), msq -> q[32], mu^2 (sbuf), var -> q[64],
                # sd -> q[96]
                muxr = rowp.tile([33, 512], F32, tag="mux", name="muxr",
                                 bufs=2)
                mu_sb = muxr[0:1, :]
                mu2_sb = muxr[32:33, :]
                rs_row = rowp.tile([1, 512], F32R, tag="rsr", name="rsrow",
                                   bufs=2)
                mu_row = rowp.tile([1, 512], F32, tag="rcp", name="murow",
                                   bufs=2)
                nc.vector.tensor_scalar(out=mu_sb, in0=s[0:1, :],
                                        scalar1=1.0 / E, scalar2=None,
                                        op0=ALU.mult)
                nc.vector.tensor_tensor(out=mu2_sb, in0=mu_sb,
                                        in1=mu_sb, op=ALU.mult)
                nc.vector.scalar_tensor_tensor(
                    out=q[64:65, :], in0=q[0:1, :], scalar=1.0 / E,
                    in1=mu2_sb, op0=ALU.mult, op1=ALU.subtract)
                nc.scalar.activation(out=q[96:97, :], in_=q[64:65, :],
                                     func=AF.Sqrt, bias=eps_t[0:1, :], scale=1.0)
                with nc.allow_low_precision(reason="f32r feed for bcast mm"):
                    nc.vector.reciprocal(out=rs_row, in_=q[96:97, :])
                nc.vector.scalar_tensor_tensor(
                    out=mu_row, in0=s[0:1, :], scalar=1.0 / E,
                    in1=rs_row.bitcast(F32), op0=ALU.mult, op1=ALU.mult)

                rs_b = psc.tile([128, 512], F32, tag="sc", name="rsb")
                nc.tensor.matmul(rs_b[:, :], ones_row, rs_row,
                                 start=True, stop=True)
                mu_b = rcbp.tile([128, 512], F32, tag="rcb", name="mub",
                                 bufs=2)
                nc.gpsimd.partition_broadcast(mu_b, mu_row)
                rs_bc.append(rs_b)
                mu_bc.append(mu_b)
            for c in range(KC):
                src = tile_chunks[c]
                dst = out_chunks[c]
                for t in range(nt):
                    sl = slice(t * 512, (t + 1) * 512)
                    d0 = dst[:, sl]
                    d0r = d0.bitcast(F32) if d0.dtype == F32R else d0
                    nc.vector.tensor_tensor(out=d0,
                                            in0=src[:, sl].bitcast(F32),
                                            in1=rs_bc[t][:, :], op=ALU.mult)
                    nc.gpsimd.tensor_tensor(out=d0,
                                            in0=d0r,
                                            in1=mu_bc[t][:, :], op=ALU.subtract)
            return

        def make_v(v_tile, src_fn, wv_dram, bias_b, masked):
            wvh2 = wvp.tile([128, 2, KC, 384], F32R, tag="wv", name="wvh2")
            for half in range(2):
                nc.sync.dma_start(out=wvh2[:, half, :, :],
                                  in_=wv_dram[:, half, :, :])
            for h0 in (0, 6):
                wvh = wvh2[:, h0 // 6]
                c0 = h0 * 64
                for tt in range(NKT):
                    ps = psl.tile([128, 512], F32, tag="lin", name="vps")
                    for kc in range(KC):
                        nc.tensor.matmul(ps[:, 0:384],
                                         src_fn(kc)[:, tt * 128:(tt + 1) * 128],
                                         wvh[:, kc, :],
                                         start=(kc == 0), stop=(kc == KC - 1))
                    vrow = v_tile[:, tt, :].rearrange("p (h c) -> p h c", c=65)
                    nc.vector.tensor_tensor(
                        out=vrow[:, h0:h0 + 6, 0:64],
                        in0=ps[:, 0:384].rearrange("p (h c) -> p h c", c=64),
                        in1=bias_b[:, c0:c0 + 384].rearrange(
                            "p (h c) -> p h c", c=64),
                        op=ALU.add)
                    if masked:
                        nc.vector.tensor_scalar(
                            out=v_tile[:, tt, h0 * 65:(h0 + 6) * 65],
                            in0=v_tile[:, tt, h0 * 65:(h0 + 6) * 65].bitcast(F32),
                            scalar1=mm_sb[:, tt:tt + 1], scalar2=None,
                            op0=ALU.mult)

        def init_ones_cols(v_tile):
            for tt in range(NKT):
                vrow = v_tile[:, tt, :].rearrange("p (h c) -> p h c", c=65)
                nc.vector.tensor_copy(
                    out=vrow[:, :, 64:65],
                    in_=onesf[:, :].rearrange("p (h o) -> p h o", o=1))

        def finish_head(av, out_sb, c, hh):
            rcp = rowp.tile([1, 512], F32, tag="rcp", name="rcp", bufs=2)
            nc.vector.reciprocal(out=rcp, in_=av[64:65, :])
            rcb = rcbp.tile([128, 512], F32, tag="rcb", name="rcb")
            nc.gpsimd.partition_broadcast(rcb, rcp)
            nc.vector.tensor_tensor(out=out_sb[hh * 64:(hh + 1) * 64, c, :],
                                    in0=av[0:64, :], in1=rcb[0:64, :],
                                    op=ALU.mult)

        def attention(kchunk_fn, v_tile, q_tile, out_sb, scale, out_mem=None):
            for c in range(KC):
                kch = kchunk_fn(c)
                av = [psa.tile([128, 512], F32, tag="av", name="av")
                      for _ in range(2)]
                for kt in range(NKT):
                    scp = []
                    for hh in range(2):
                        off = hh * 64
                        sc = psc.tile([128, 512], F32, tag="sc", name="sc")
                        nc.tensor.matmul(
                            sc[:, :],
                            kch[off:off + 64, kt * 128:(kt + 1) * 128],
                            q_tile[off:off + 64, c, :], start=True, stop=True)
                        scp.append(sc)
                    for hh in range(2):
                        h = 2 * c + hh
                        pt = ptp.tile([128, 512], F32R, tag="pt", name="pt")
                        nc.scalar.activation(out=pt, in_=scp[hh][:, :],
                                             func=AF.Exp, scale=scale)
                        nc.tensor.matmul(av[hh][0:65, :],
                                         v_tile[:, kt, h * 65:(h + 1) * 65],
                                         pt, start=(kt == 0), stop=(kt == NKT - 1))
                for hh in range(2):
                    finish_head(av[hh], out_sb, c, hh)
                if out_mem is not None:
                    scm = []
                    for hh in range(2):
                        off = hh * 64
                        sm = psc.tile([128, 512], F32, tag="sc", name="scm")
                        nc.tensor.matmul(sm[0:M, :], mk_sb[off:off + 64, c, :],
                                         q_tile[off:off + 64, c, :],
                                         start=True, stop=True)
                        scm.append(sm)
                    for hh in range(2):
                        h = 2 * c + hh
                        pmt = ptp.tile([128, 512], F32R, tag="pt", name="pmt")
                        nc.scalar.activation(out=pmt[0:M, :],
                                             in_=scm[hh][0:M, :],
                                             func=AF.Exp, scale=1.0)
                        avm = psa.tile([128, 512], F32, tag="av", name="avm")
                        nc.tensor.matmul(avm[0:65, :],
                                         mv_sb[:, h * 65:(h + 1) * 65],
                                         pmt[0:M, :], start=True, stop=True)
                        finish_head(avm, out_mem, c, hh)

        # ---- persistent tiles ----
        v_sb = vsbp.tile([128, NKT, H * 65], F32R, tag="vsb")
        init_ones_cols(v_sb)

        # ======== phase A: xhat (in-place LN; DMA already emitted) ========
        stats_norm([xhat[:, c, :] for c in range(KC)], S,
                   [xhat[:, c, :] for c in range(KC)])
        tapc("d_xhat", [xhat[:, c, :] for c in range(KC)])
        half0 = 0  # query half offset handled on host; xqh = xhat columns

        def xqh(kc):
            return xhat[:, kc, 0:Q]

        # NOTE: host passes xT with this core's query half in columns [0:Q]
        # (i.e. xT is rolled so the q-half comes first). See kernel().

        # ======== phase B: v, q, then self+memory attention ========
        make_v(v_sb, lambda kc: xhat[:, kc, :], wv_self, bvb_s, masked=False)
        qT = n12p.tile([128, KC, Q], F32R, tag="A", name="qT")
        linear(lambda ft: qT[:, ft, :], xqh, "qk", KC, KC, Q, BC_Q, AF.Identity)
        tapc("d_qT", [qT[:, c, :] for c in range(KC)])

        def self_kchunk(c):
            kt = keyp.tile([128, S], F32R, tag="keyc", name="kch")
            tiles = linear_ps(lambda kc: xhat[:, kc, :], "qk", KC, c, S,
                              fcol0=KC)
            for t in range(2):
                nc.vector.tensor_scalar(
                    out=kt[:, t * 512:(t + 1) * 512], in0=tiles[t][:, :],
                    scalar1=bc[:, BC_K + c:BC_K + c + 1], scalar2=None,
                    op0=ALU.add)
            return kt

        aat = n12p.tile([128, KC, Q], F32R, tag="B", name="aat")
        amem = n12p.tile([128, KC, Q], F32R, tag="C", name="amem")
        attention(self_kchunk, v_sb, qT, aat, 1.0, out_mem=amem)
        tapc("d_aat", [aat[:, c, :] for c in range(KC)])
        tapc("d_amem", [amem[:, c, :] for c in range(KC)])

        # ======== phase B2 (emit early): enc0 DMA only ========
        ehat0 = bigp.tile([128, KC, 1024], F32R, tag="big", name="ehat0")
        ln_dma([ehat0[:, c, :] for c in range(KC)],
               lambda c: encT[0, c * 128:(c + 1) * 128, :])

        # ======== phase C: gate + attn_proj + residual, LN, qe ========
        aN = n12p.tile([128, KC, Q], F32R, tag="D", name="aN")

        def gate_rhs(kc):
            return aat[:, kc, :] if kc < KC else amem[:, kc - KC, :]

        for ft in range(KC):
            tiles = linear_ps(gate_rhs, "ma", 12, ft, Q)
            al = rcbp.tile([128, 512], F32, tag="rcb", name="al")
            nc.scalar.activation(out=al, in_=tiles[0][:, :], func=AF.Sigmoid,
                                 bias=bc[:, BC_MA + ft:BC_MA + ft + 1],
                                 scale=1.0)
            eng = nc.vector if ft % 2 == 0 else nc.gpsimd
            d = tmpp.tile([128, 512], F32, tag="d", name="d")
            eng.tensor_tensor(out=d, in0=aat[:, ft, :].bitcast(F32),
                              in1=amem[:, ft, :].bitcast(F32),
                              op=ALU.subtract)
            eng.tensor_tensor(out=d, in0=al, in1=d, op=ALU.mult)
            eng.tensor_tensor(out=aN[:, ft, :],
                              in0=amem[:, ft, :].bitcast(F32),
                              in1=d, op=ALU.add)
        aT = n12p.tile([128, KC, Q], F32R, tag="E", name="aT")
        for ft in range(KC):
            tiles = linear_ps(lambda kc: aN[:, kc, :], "proj", KC, ft, Q)
            xq_c = ptp.tile([128, 512], F32R, tag="pt", name="xqc")
            nc.sync.dma_start(out=xq_c, in_=xqT[ft * 128:(ft + 1) * 128, :])
            nc.vector.scalar_tensor_tensor(
                out=aT[:, ft, :], in0=tiles[0][:, :],
                scalar=bc[:, BC_PROJ + ft:BC_PROJ + ft + 1],
                in1=xq_c.bitcast(F32), op0=ALU.add, op1=ALU.add)
        tapc("d_aT", [aT[:, c, :] for c in range(KC)])

        # enc0 LN stats+normalize (data already resident; overlaps with C)
        stats_norm([ehat0[:, c, :] for c in range(KC)], SE,
                   [ehat0[:, c, :] for c in range(KC)])
        # enc0 V production early: fills PE during hah-LN/qeT serial chains
        make_v(v_sb, lambda kc: ehat0[:, kc, :], wv_enc, bvb_e, masked=True)

        hah = n12p.tile([128, KC, Q], F32R, tag="C", name="hah")
        stats_norm([aT[:, c, :] for c in range(KC)], Q,
                   [hah[:, c, :] for c in range(KC)])
        tapc("d_hah", [hah[:, c, :] for c in range(KC)])
        qeT = n12p.tile([128, KC, Q], F32R, tag="A", name="qeT")
        linear(lambda ft: qeT[:, ft, :], lambda kc: hah[:, kc, :],
               "q", KC, KC, Q, BC_FCQ, AF.Identity)

        # ======== phase D: two cross-attentions ========
        e1p = None
        asum = None
        ehat1 = None
        for e in range(2):
            if e == 0:
                ehat = ehat0
            else:
                ehat = ehat1
                make_v(v_sb, lambda kc, _eh=ehat: _eh[:, kc, :], wv_enc,
                       bvb_e, masked=True)

            def enc_kchunk(c, _eh=ehat):
                kt = keyp.tile([128, SE], F32R, tag="keyc", name="kche")
                tiles = linear_ps(lambda kc: _eh[:, kc, :], "k", KC, c, SE)
                for t in range(2):
                    nc.vector.tensor_scalar(
                        out=kt[:, t * 512:(t + 1) * 512], in0=tiles[t][:, :],
                        scalar1=bc[:, BC_FCK + c:BC_FCK + c + 1], scalar2=None,
                        op0=ALU.add)
                return kt

            eaT = n12p.tile([128, KC, Q], F32R, tag="B", name="eaT")
            attention(enc_kchunk, v_sb, qeT, eaT, 0.125)
            if e == 0:
                tapc("d_eaT", [eaT[:, c, :] for c in range(KC)])
                # prefetch + normalize enc1 while e0 epilogue runs
                ehat1 = bigp.tile([128, KC, 1024], F32R, tag="big",
                                  name="ehat1")
                ln_dma([ehat1[:, c, :] for c in range(KC)],
                       lambda c: encT[1, c * 128:(c + 1) * 128, :])
                stats_norm([ehat1[:, c, :] for c in range(KC)], SE,
                           [ehat1[:, c, :] for c in range(KC)])

            ep_o = n12p.tile([128, KC, Q], F32R, tag="D", name="ep_o")
            linear(lambda ft: ep_o[:, ft, :], lambda kc: eaT[:, kc, :],
                   "ep", KC, KC, Q, BC_EP, AF.Identity)
            bcol0 = BC_A1 if e == 0 else BC_A2

            def alpha_rhs(kc, _ep=ep_o):
                return aT[:, kc, :] if kc < KC else _ep[:, kc - KC, :]

            if e == 0:
                e1p = n12p.tile([128, KC, Q], F32R, tag="C", name="e1p")
            else:
                asum = n12p.tile([128, KC, Q], F32R, tag="A", name="asum")
            for ft in range(KC):
                tiles = linear_ps(alpha_rhs, "a1" if e == 0 else "a2", 12,
                                  ft, Q)
                al = rcbp.tile([128, 512], F32, tag="rcb", name="alE")
                nc.scalar.activation(out=al, in_=tiles[0][:, :],
                                     func=AF.Sigmoid,
                                     bias=bc[:, bcol0 + ft:bcol0 + ft + 1],
                                     scale=1.0)
                eng = nc.vector if ft % 2 == 0 else nc.gpsimd
                d = tmpp.tile([128, 512], F32, tag="d", name="dE")
                eng.tensor_tensor(out=d, in0=aT[:, ft, :].bitcast(F32),
                                  in1=ep_o[:, ft, :].bitcast(F32),
                                  op=ALU.subtract)
                eng.tensor_tensor(out=d, in0=al, in1=d, op=ALU.mult)
                if e == 0:
                    eng.tensor_tensor(out=e1p[:, ft, :],
                                      in0=ep_o[:, ft, :].bitcast(F32),
                                      in1=d, op=ALU.add)
                else:
                    eng.tensor_tensor(out=d,
                                      in0=ep_o[:, ft, :].bitcast(F32),
                                      in1=d, op=ALU.add)
                    eng.tensor_tensor(out=asum[:, ft, :],
                                      in0=e1p[:, ft, :].bitcast(F32),
                                      in1=d, op=ALU.add)

        # ======== phase E/F: MLP + final residual ========
        tapc("d_asum", [asum[:, c, :] for c in range(KC)])
        hm2 = n12p.tile([128, KC, Q], BF16, tag="C", name="hm2")
        stats_norm([asum[:, c, :] for c in range(KC)], Q,
                   [hm2[:, c, :] for c in range(KC)])
        mstage = n12p.tile([128, KC, Q], F32, tag="D", name="mstage")
        for mh in range(2):
            mT = bigp.tile([128, 12, Q], BF16, tag="big", name="mT")
            linear(lambda ft: mT[:, ft, :], lambda kc: hm2[:, kc, :],
                   "fc", KC, 12, Q, BC_FC + 12 * mh, AF.Gelu_apprx_tanh,
                   fcol0=12 * mh)
            for ft in range(KC):
                wt = wslice("pj", 12, ft, kc0=12 * mh, dt=BF16)
                ps = psl.tile([128, 512], F32, tag="lin", name="pjps")
                for kc in range(12):
                    nc.tensor.matmul(ps[:, :], wt[:, kc, :], mT[:, kc, :],
                                     start=(kc == 0), stop=(kc == 11))
                if mh == 0:
                    nc.scalar.activation(out=mstage[:, ft, :], in_=ps[:, :],
                                         func=AF.Identity,
                                         bias=bc[:, BC_PJ + ft:BC_PJ + ft + 1],
                                         scale=1.0)
                else:
                    t = tmpp.tile([128, 512], F32, tag="d", name="mo")
                    nc.vector.scalar_tensor_tensor(
                        out=t, in0=asum[:, ft, :].bitcast(F32),
                        scalar=float(1.0 / np.sqrt(2.0)), in1=ps[:, :],
                        op0=ALU.mult, op1=ALU.add)
                    ot = tmpp.tile([128, 512], F32, tag="d", name="ot")
                    nc.vector.tensor_tensor(out=ot, in0=t,
                                            in1=mstage[:, ft, :], op=ALU.add)
                    nc.sync.dma_start(out=outT[ft * 128:(ft + 1) * 128, :],
                                      in_=ot)


_NC_CACHE = None


def _get_nc():
    global _NC_CACHE
    if _NC_CACHE is None:
        _NC_CACHE = build_program()
    return _NC_CACHE


def _pack_bias_cols(seg_biases):
    bcols = np.zeros((128, NB), np.float32)
    for col0, b in seg_biases:
        nf = b.shape[0] // 128
        bcols[:, col0:col0 + nf] = b.reshape(nf, 128).T
    return bcols


def _pack_w(w):
    """[K, F] -> [128, F//128, K//128, 128]"""
    K, F = w.shape
    return np.ascontiguousarray(
        w.reshape(K // 128, 128, F // 128, 128).transpose(1, 2, 0, 3))


def _pack_w16(w):
    import ml_dtypes
    return _pack_w(w).astype(ml_dtypes.bfloat16)


def _pack_wv(w):
    """[E, E] -> [128, 2, KC, 384] (v-production halves)"""
    return np.ascontiguousarray(
        w.reshape(KC, 128, 2, 384).transpose(1, 2, 0, 3))


def kernel(x, encoder_features, mask_encoder, ln1_g, ln1_b, ln2_g, ln2_b,
           c_attn_w, c_attn_b, attn_proj_w, attn_proj_b,
           memory_features, mem_attn_w, mem_attn_b, mem_alpha_w, mem_alpha_b,
           fcq_w, fcq_b, fck_w, fck_b, fcv_w, fcv_b, enc_proj_w, enc_proj_b,
           fc_alpha1_w, fc_alpha1_b, fc_alpha2_w, fc_alpha2_b,
           mlp_fc_w, mlp_fc_b, mlp_proj_w, mlp_proj_b):
    f32 = np.float32
    x = np.asarray(x, f32)
    encoder_features = np.asarray(encoder_features, f32)

    # ---- fold LN gains/biases into consumer weights ----
    g1 = np.asarray(ln1_g, f32); b1 = np.asarray(ln1_b, f32)
    g2 = np.asarray(ln2_g, f32); b2 = np.asarray(ln2_b, f32)

    def fold(w, b, g, lb):
        w = np.asarray(w, f32); b = np.asarray(b, f32)
        return (w * g[:, None]).astype(f32), (lb @ w + b).astype(f32)

    w_qkv, b_qkv = fold(c_attn_w, c_attn_b, g1, b1)
    w_fcq, b_fcq = fold(fcq_w, fcq_b, g1, b1)
    w_fck, b_fck = fold(fck_w, fck_b, g1, b1)
    w_fcv, b_fcv = fold(fcv_w, fcv_b, g1, b1)
    w_mfc, b_mfc = fold(mlp_fc_w, mlp_fc_b, g2, b2)

    # ---- memory slots (batch independent) ----
    mem = (np.asarray(memory_features, f32)[0] @ np.asarray(mem_attn_w, f32)
           + np.asarray(mem_attn_b, f32))          # [M, 2E]
    mk = mem[:, :E].reshape(M, H, D)
    mv = mem[:, E:].reshape(M, H, D)
    mkT = np.zeros((128, KC, M), f32)
    mvA = np.zeros((M, H * 65), f32)
    for h in range(H):
        c, off = divmod(h, 2)
        mkT[off * 64:(off + 1) * 64, c, :] = mk[:, h, :].T
        mvA[:, h * 65:h * 65 + 64] = mv[:, h, :]
        mvA[:, h * 65 + 64] = 1.0

    bcols = _pack_bias_cols([
        (BC_Q, b_qkv[0:E]), (BC_K, b_qkv[E:2 * E]),
        (BC_PROJ, np.asarray(attn_proj_b, f32)),
        (BC_MA, np.asarray(mem_alpha_b, f32)),
        (BC_FCQ, b_fcq), (BC_FCK, b_fck),
        (BC_EP, np.asarray(enc_proj_b, f32)),
        (BC_A1, np.asarray(fc_alpha1_b, f32)),
        (BC_A2, np.asarray(fc_alpha2_b, f32)),
        (BC_FC, b_mfc), (BC_PJ, np.asarray(mlp_proj_b, f32)),
    ])

    keep = (~np.asarray(mask_encoder, bool)[:, 0, 0, :]).astype(f32)  # [B, SE]

    common = dict(
        mkT=mkT, mvA=mvA,
        wp_qk=_pack_w(np.ascontiguousarray(w_qkv[:, 0:2 * E])),
        wv_self=_pack_wv(np.ascontiguousarray(w_qkv[:, 2 * E:3 * E])),
        wp_proj=_pack_w(np.asarray(attn_proj_w, f32)),
        wp_ma=_pack_w(np.asarray(mem_alpha_w, f32)),
        wp_q=_pack_w(w_fcq), wp_k=_pack_w(w_fck),
        wv_enc=_pack_wv(w_fcv),
        wp_ep=_pack_w(np.asarray(enc_proj_w, f32)),
        wp_a1=_pack_w(np.asarray(fc_alpha1_w, f32)),
        wp_a2=_pack_w(np.asarray(fc_alpha2_w, f32)),
        wp_fc=_pack_w16(w_mfc), wp_pj=_pack_w16(np.asarray(mlp_proj_w, f32)),
        bcols=bcols,
        bv_self=np.ascontiguousarray(b_qkv[2 * E:3 * E]),
        bv_enc=b_fcv,
    )

    in_maps = []
    for core in range(8):
        b, half = divmod(core, 2)
        xTb = np.ascontiguousarray(x[b].T)                       # [E, S]
        # roll so this core's query half occupies columns [0:Q]
        xroll = np.ascontiguousarray(np.roll(xTb, -half * Q, axis=1))
        m = dict(common)
        m["xT"] = xroll
        m["xqT"] = np.ascontiguousarray(xTb[:, half * Q:(half + 1) * Q])
        m["encT"] = np.ascontiguousarray(encoder_features[b].transpose(0, 2, 1))
        m["maskmul"] = np.ascontiguousarray(keep[b].reshape(NKT, 128).T)
        in_maps.append(m)

    nc = _get_nc()
    res = run_bass_kernel_spmd(nc, in_maps, core_ids=list(range(8)))

    global _LAST_IN_MAPS
    _LAST_IN_MAPS = in_maps

    y = np.empty((B, S, E), f32)
    for core in range(8):
        b, half = divmod(core, 2)
        y[b, half * Q:(half + 1) * Q, :] = res.results[core]["outT"].T
    return y


_LAST_IN_MAPS = None


def profile_exec_ns(n_hot=12, n_cold=2, n_trials=18):
    """Estimate per-invocation device time by timing pipelined repeats of the
    jitted 8-core executable with device-resident inputs. The axon/PJRT
    dispatch path adds large, variable per-call overhead, so the marginal
    rate is measured several times and the minimum (least-noise draw) is
    reported."""
    import time
    import jax
    from jax.sharding import Mesh, PartitionSpec
    from jax.experimental.shard_map import shard_map
    import concourse.mybir as mybir_
    from concourse import bass2jax

    if _LAST_IN_MAPS is None:
        return None
    nc = _get_nc()
    in_maps = _LAST_IN_MAPS
    n_cores = 8
    bass2jax.install_neuronx_cc_hook()

    in_names, out_names, out_avals, zero_outs = [], [], [], []
    partition_name = nc.partition_id_tensor.name if nc.partition_id_tensor else None
    for alloc in nc.m.functions[0].allocations:
        if not isinstance(alloc, mybir_.MemoryLocationSet):
            continue
        name = alloc.memorylocations[0].name
        if alloc.kind == "ExternalInput":
            if name != partition_name:
                in_names.append(name)
        elif alloc.kind == "ExternalOutput":
            out_avals.append(jax.core.ShapedArray(
                tuple(alloc.tensor_shape), mybir_.dt.np(alloc.dtype)))
            zero_outs.append(np.zeros(tuple(alloc.tensor_shape),
                                      mybir_.dt.np(alloc.dtype)))
            out_names.append(name)
    n_params = len(in_names)
    n_outs = len(out_avals)
    all_in_names = in_names + out_names + ([partition_name] if partition_name else [])
    donate = tuple(range(n_params, n_params + n_outs))

    def _body(*args):
        operands = list(args)
        if partition_name is not None:
            operands.append(bass2jax.partition_id_tensor())
        return tuple(bass2jax._bass_exec_p.bind(
            *operands, out_avals=tuple(out_avals), in_names=tuple(all_in_names),
            out_names=tuple(out_names), lowering_input_output_aliases=(),
            sim_require_finite=True, sim_require_nnan=True, nc=nc))

    devices = jax.devices()[:n_cores]
    mesh = Mesh(np.asarray(devices), ("core",))
    fn = jax.jit(shard_map(_body, mesh=mesh,
                           in_specs=(PartitionSpec("core"),) * (n_params + n_outs),
                           out_specs=(PartitionSpec("core"),) * n_outs,
                           check_rep=False),
                 donate_argnums=donate, keep_unused=True)
    sh = jax.sharding.NamedSharding(mesh, PartitionSpec("core"))
    concat_in = [jax.device_put(
        np.concatenate([np.asarray(in_maps[c][nm]) for c in range(n_cores)], 0), sh)
        for nm in in_names]

    def zeros():
        return [jax.device_put(
            np.zeros((n_cores * z.shape[0], *z.shape[1:]), z.dtype), sh)
            for z in zero_outs]

    def run(n):
        o = tuple(zeros())
        o = fn(*concat_in, *o)
        jax.block_until_ready(o)
        t0 = time.perf_counter()
        for _ in range(n):
            o = fn(*concat_in, *o)
        jax.block_until_ready(o)
        return time.perf_counter() - t0

    best = None
    fallback = None
    for _i in range(n_trials):
        if _i:
            time.sleep(0.25)
        tc = run(n_cold)
        th = run(n_hot)
        per = (th - tc) / (n_hot - n_cold)
        print(f"pipelined wall: {n_cold} calls {tc*1e3:.2f} ms, "
              f"{n_hot} calls {th*1e3:.2f} ms -> per-call {per*1e6:.0f} us")
        fb = th / n_hot
        fallback = fb if fallback is None else min(fallback, fb)
        if per > 0:
            best = per if best is None else min(best, per)
    if best is None:
        best = fallback
    return int(best * 1e9)
